# Initial kernel scaffold
#
"""Pairwise Euclidean distance kernel for Trainium2 (8 NeuronCores, SPMD).

Problem: mapping [8192, 256] f32 -> out [8192, 8192] f32 where
out[i, j] = ||mapping[i] - mapping[j]||_2, via the GEMM identity
d2 = ||x_i||^2 + ||x_j||^2 - 2 <x_i, x_j>.

Sharding: row-block of the output per core. Core c computes rows
[c*1024, (c+1)*1024) against all 8192 columns. To keep one SPMD program
with static addressing, each core's inputs are rotated by c*1024 (rows of
the natural layout / columns of the transposed layout); the host un-rotates
each core's output columns afterwards.

Per-core on-device pipeline:
  - inputs: mt [256, 8192] f16 (x^T, rotated), nat [8192, 256] f16 (x,
    rotated), eye [128, 128] f16 (transpose identity)
  - sq_j = sum_k x~[j,k]^2 in f32 on DVE (square + 3D reduce), where x~ is
    the f16-rounded input; using the same rounded values for the gram and
    for sq makes the diagonal cancel to ~1e-4.
  - The -0.5*sq_j row is split hi/lo into two f16 rows (exact to ~2^-22) and
    folded into the PSUM accumulation as a K=2 rank-1 matmul with an all-ones
    stationary operand: psum = gram - 0.5*sq_j.
  - ACT computes sqrt(-2*psum + sq_i) with per-partition bias sq_i, reading
    PSUM directly. d2 can only go negative (fp rounding) where the true
    distance is 0, i.e. the i==j block, so a [128,128] tensor_scalar_min
    clamp (psum <= 0.5*sq_i) before ACT protects exactly that block.
  - matmul dtype f16: PE multiplies f16 exactly into f32 PSUM; the only
    error vs the f32 reference is the input rounding (~2e-4 relative).
"""

import sys

try:
    import concourse.bass as _probe  # noqa: F401
except ImportError:
    sys.path.insert(0, "/opt/trn_rl_repo")

import numpy as np

import concourse.bacc as bacc
import concourse.mybir as mybir
from concourse import tile
from concourse.bass_utils import run_bass_kernel_spmd

N = 8192          # number of points
D = 256           # feature dim
NCORES = 8
RPC = N // NCORES  # 1024 rows per core
RT = RPC // 128    # 8 row-tiles per core
JCHUNK = 2048      # output chunk width (4 PSUM banks)
NJC = N // JCHUNK  # 4 chunks
NSUB = JCHUNK // 512  # 4 matmul sub-tiles per chunk
NGRP = 8           # sq reduction groups (8 tiles of 128 rows each)

F16 = mybir.dt.float16
F32 = mybir.dt.float32


def _build_nc():
    nc = bacc.Bacc(None, target_bir_lowering=False)
    mt_d = nc.dram_tensor("mt", [D, N], F16, kind="ExternalInput")
    nat_d = nc.dram_tensor("nat", [N, D], F16, kind="ExternalInput")
    eye_d = nc.dram_tensor("eye", [128, 128], F16, kind="ExternalInput")
    out_d = nc.dram_tensor("out", [RPC, N], F32, kind="ExternalOutput")

    with tile.TileContext(nc) as tc:
        with (
            tc.tile_pool(name="big", bufs=1) as big,
            tc.tile_pool(name="work", bufs=2) as work,
            tc.tile_pool(name="stage", bufs=4) as stage_pool,
            tc.tile_pool(name="ps", bufs=2, space="PSUM") as psum,
        ):
            # --- persistent SBUF tensors ---
            mt0 = big.tile([128, N], F16, tag="mt0")
            mt1 = big.tile([128, N], F16, tag="mt1")
            nc.sync.dma_start(mt0[:], mt_d[0:128, :])
            nc.sync.dma_start(mt1[:], mt_d[128:256, :])
            eye = big.tile([128, 128], F16, tag="eye")
            nc.sync.dma_start(eye[:], eye_d[:])
            ones2 = big.tile([2, 128], F16, tag="ones2")
            nc.vector.memset(ones2[:], 1.0)
            sq_tiles = big.tile([128, 64], F32, tag="sq_tiles")
            sq_flat = big.tile([2, N], F16, tag="sq_flat")

            # --- sq_j = sum over D of nat^2, in tile layout [p, t] ---
            nat_g = nat_d.rearrange("(g t p) d -> g p t d", g=NGRP, p=128)
            for g in range(NGRP):
                gt = work.tile([128, 8, 256], F16, tag="natg")
                nc.sync.dma_start(gt[:], nat_g[g])
                msq = work.tile([128, 8, 256], F32, tag="msq")
                nc.vector.tensor_mul(msq[:], gt[:], gt[:])
                nc.vector.reduce_sum(
                    sq_tiles[:, g * 8:(g + 1) * 8].unsqueeze(2),
                    msq[:],
                    axis=mybir.AxisListType.X,
                )

            # 0.5 * sq for the diagonal clamp (own rows = tiles 0..7)
            half_own = big.tile([128, 8], F32, tag="half_own")
            nc.vector.tensor_scalar_mul(half_own[:], sq_tiles[:, 0:8], 0.5)

            # hi/lo f16 split of -0.5*sq
            mh32 = work.tile([128, 64], F32, tag="mh32")
            nc.vector.tensor_scalar_mul(mh32[:], sq_tiles[:], -0.5)
            hi16 = work.tile([128, 64], F16, tag="hi16")
            nc.vector.tensor_copy(hi16[:], mh32[:])
            hi32 = work.tile([128, 64], F32, tag="hi32")
            nc.vector.tensor_copy(hi32[:], hi16[:])
            lo16 = work.tile([128, 64], F16, tag="lo16")
            nc.vector.tensor_sub(lo16[:], mh32[:], hi32[:])

            # transpose [128, 64] -> [64, 128] on PE, then flatten to [1, N]
            for row, src in ((0, hi16), (1, lo16)):
                pt = psum.tile([64, 128], F32, tag="ps")
                nc.tensor.transpose(pt[:], src[:], eye[:])
                st = work.tile([64, 128], F16, tag="sqT")
                nc.vector.tensor_copy(st[:], pt[:])
                nc.sync.dma_start(
                    sq_flat[row:row + 1, :].rearrange("o (t i) -> o t i", t=64),
                    st[:],
                )

            # --- main loop: 8 row-tiles x 4 chunks of 2048 ---
            for r in range(RT):
                lhs0 = mt0[:, r * 128:(r + 1) * 128]
                lhs1 = mt1[:, r * 128:(r + 1) * 128]
                for jc in range(NJC):
                    ps = psum.tile([128, JCHUNK], F32, tag="ps")
                    for s in range(NSUB):
                        j0 = jc * JCHUNK + s * 512
                        o = ps[:, s * 512:(s + 1) * 512]
                        nc.tensor.matmul(o, lhs0, mt0[:, j0:j0 + 512],
                                         start=True, stop=False)
                        nc.tensor.matmul(o, lhs1, mt1[:, j0:j0 + 512],
                                         start=False, stop=False)
                        nc.tensor.matmul(o, ones2[:], sq_flat[:, j0:j0 + 512],
                                         start=False, stop=True)
                    if jc == 0:
                        # clamp the i==j block so d2 >= 0 there
                        dg = ps[:, r * 128:(r + 1) * 128]
                        nc.vector.tensor_scalar_min(dg, dg, half_own[:, r:r + 1])
                    out_t = stage_pool.tile([128, JCHUNK], F32, tag="stage")
                    nc.scalar.activation(
                        out_t[:], ps[:],
                        mybir.ActivationFunctionType.Sqrt,
                        bias=sq_tiles[:, r:r + 1], scale=-2.0,
                    )
                    nc.sync.dma_start(
                        out_d[r * 128:(r + 1) * 128,
                              jc * JCHUNK:(jc + 1) * JCHUNK],
                        out_t[:],
                    )

    nc.compile()
    return nc


_NC_CACHE = None


def _get_nc():
    global _NC_CACHE
    if _NC_CACHE is None:
        _NC_CACHE = _build_nc()
    return _NC_CACHE


def kernel(mapping: np.ndarray, **_kwargs) -> np.ndarray:
    mapping = np.asarray(mapping, dtype=np.float32)
    assert mapping.shape == (N, D)
    xh = mapping.astype(np.float16)
    eye = np.eye(128, dtype=np.float16)

    in_maps = []
    for c in range(NCORES):
        natc = np.ascontiguousarray(np.roll(xh, -c * RPC, axis=0))
        mtc = np.ascontiguousarray(natc.T)
        in_maps.append({"mt": mtc, "nat": natc, "eye": eye})

    nc = _get_nc()
    res = run_bass_kernel_spmd(nc, in_maps, core_ids=list(range(NCORES)))

    out = np.empty((N, N), dtype=np.float32)
    for c in range(NCORES):
        out[c * RPC:(c + 1) * RPC] = np.roll(res.results[c]["out"], c * RPC, axis=1)
    return out


if __name__ == "__main__":
    rng = np.random.default_rng(0)
    x = rng.standard_normal((N, D)).astype(np.float32)
    o = kernel(mapping=x)
    print("out", o.shape, o.dtype, "sample", o[0, :4], "diag", np.abs(np.diag(o)).max())


# revision 5
# speedup vs baseline: 1.1305x; 1.1305x over previous
"""Pairwise Euclidean distance kernel for Trainium2 (8 NeuronCores, SPMD).

Problem: mapping [8192, 256] f32 -> out [8192, 8192] f32 where
out[i, j] = ||mapping[i] - mapping[j]||_2, via the GEMM identity
d2 = ||x_i||^2 + ||x_j||^2 - 2 <x_i, x_j>.

Sharding: row-block of the output per core. Core c computes rows
[c*1024, (c+1)*1024) against all 8192 columns. To keep one SPMD program
with static addressing, each core's inputs are rotated by c*1024 (rows of
the natural layout / columns of the transposed layout); the host un-rotates
each core's output columns afterwards.

Per-core on-device pipeline:
  - inputs: mt [256, 8192] f16 (x^T, rotated), nat [8192, 256] f16 (x,
    rotated), eye [128, 128] f16 (transpose identity)
  - sq_j = sum_k x~[j,k]^2 in f32 on DVE (square + 3D reduce), where x~ is
    the f16-rounded input; using the same rounded values for the gram and
    for sq makes the diagonal cancel to ~1e-4.
  - The -0.5*sq_j row is split hi/lo into two f16 rows (exact to ~2^-22) and
    folded into the PSUM accumulation as a K=2 rank-1 matmul with an all-ones
    stationary operand: psum = gram - 0.5*sq_j.
  - ACT computes sqrt(-2*psum + sq_i) with per-partition bias sq_i, reading
    PSUM directly. d2 can only go negative (fp rounding) where the true
    distance is 0, i.e. the i==j block, so a [128,128] tensor_scalar_min
    clamp (psum <= 0.5*sq_i) before ACT protects exactly that block.
  - matmul dtype f16: PE multiplies f16 exactly into f32 PSUM; the only
    error vs the f32 reference is the input rounding (~2e-4 relative).
"""

import sys

try:
    import concourse.bass as _probe  # noqa: F401
except ImportError:
    sys.path.insert(0, "/opt/trn_rl_repo")

import numpy as np

import concourse.bacc as bacc
import concourse.mybir as mybir
from concourse import tile
from concourse.bass_utils import run_bass_kernel_spmd

N = 8192          # number of points
D = 256           # feature dim
NCORES = 8
RPC = N // NCORES  # 1024 rows per core
RT = RPC // 128    # 8 row-tiles per core
JCHUNK = 2048      # output chunk width (4 PSUM banks)
NJC = N // JCHUNK  # 4 chunks
NSUB = JCHUNK // 512  # 4 matmul sub-tiles per chunk
NGRP = 8           # sq reduction groups (8 tiles of 128 rows each)

F16 = mybir.dt.float16
F32 = mybir.dt.float32


def _build_nc(repeats=1):
    nc = bacc.Bacc(None, target_bir_lowering=False)
    mt_d = nc.dram_tensor("mt", [D, N], F16, kind="ExternalInput")
    nat_d = nc.dram_tensor("nat", [N, D], F16, kind="ExternalInput")
    eye_d = nc.dram_tensor("eye", [128, 128], F32, kind="ExternalInput")
    out_d = nc.dram_tensor("out", [RPC, N], F32, kind="ExternalOutput")

    with tile.TileContext(nc) as tc:
        with (
            tc.tile_pool(name="big", bufs=1) as big,
            tc.tile_pool(name="work", bufs=2) as work,
            tc.tile_pool(name="stage", bufs=4) as stage_pool,
            tc.tile_pool(name="ps", bufs=2, space="PSUM") as psum,
        ):
            for _rep in range(repeats):
                _emit_body(nc, big, work, stage_pool, psum,
                           mt_d, nat_d, eye_d, out_d)

    nc.compile()
    return nc


def _emit_body(nc, big, work, stage_pool, psum, mt_d, nat_d, eye_d, out_d):
    if True:
        if True:
            # --- persistent SBUF tensors ---
            mt0 = big.tile([128, N], F16, tag="mt0")
            mt1 = big.tile([128, N], F16, tag="mt1")
            nc.sync.dma_start(mt0[:], mt_d[0:128, :])
            nc.sync.dma_start(mt1[:], mt_d[128:256, :])
            eye = big.tile([128, 128], F32, tag="eye")
            nc.sync.dma_start(eye[:], eye_d[:])
            ones2 = big.tile([2, 128], F16, tag="ones2")
            nc.vector.memset(ones2[:], 1.0)
            sq_tiles = big.tile([128, 64], F32, tag="sq_tiles")
            sq_flat = big.tile([2, N], F16, tag="sq_flat")

            # --- sq_j = sum over D of nat^2, in tile layout [p, t] ---
            nat_g = nat_d.rearrange("(g t p) d -> g p t d", g=NGRP, p=128)
            for g in range(NGRP):
                gt = work.tile([128, 8, 256], F16, tag="natg")
                nc.sync.dma_start(gt[:], nat_g[g])
                msq = work.tile([128, 8, 256], F32, tag="msq")
                nc.vector.tensor_mul(msq[:], gt[:], gt[:])
                nc.vector.reduce_sum(
                    sq_tiles[:, g * 8:(g + 1) * 8].unsqueeze(2),
                    msq[:],
                    axis=mybir.AxisListType.X,
                )

            # 0.5 * sq for the diagonal clamp (own rows = tiles 0..7)
            half_own = big.tile([128, 8], F32, tag="half_own")
            nc.vector.tensor_scalar_mul(half_own[:], sq_tiles[:, 0:8], 0.5)

            # hi/lo f16 split of -0.5*sq (computed in f32, rounded on the
            # PSUM->SBUF copy after the transpose; hi32 reproduces the f16
            # rounding so lo is the exact remainder)
            mh32 = work.tile([128, 64], F32, tag="mh32")
            nc.vector.tensor_scalar_mul(mh32[:], sq_tiles[:], -0.5)
            hi16 = work.tile([128, 64], F16, tag="hi16")
            nc.vector.tensor_copy(hi16[:], mh32[:])
            hi32 = work.tile([128, 64], F32, tag="hi32")
            nc.vector.tensor_copy(hi32[:], hi16[:])
            lo32 = work.tile([128, 64], F32, tag="lo32")
            nc.vector.tensor_sub(lo32[:], mh32[:], hi32[:])

            # transpose [128, 64] -> [64, 128] on PE, then flatten to [1, N]
            for row, src in ((0, mh32), (1, lo32)):
                pt = psum.tile([64, 128], F32, tag="ps")
                nc.tensor.transpose(pt[:], src[:], eye[:])
                st = work.tile([64, 128], F16, tag="sqT")
                nc.vector.tensor_copy(st[:], pt[:])
                nc.sync.dma_start(
                    sq_flat[row:row + 1, :].rearrange("o (t i) -> o t i", t=64),
                    st[:],
                )

            # --- main loop: 8 row-tiles x 4 chunks of 2048 ---
            for r in range(RT):
                lhs0 = mt0[:, r * 128:(r + 1) * 128]
                lhs1 = mt1[:, r * 128:(r + 1) * 128]
                for jc in range(NJC):
                    ps = psum.tile([128, JCHUNK], F32, tag="ps")
                    for s in range(NSUB):
                        j0 = jc * JCHUNK + s * 512
                        o = ps[:, s * 512:(s + 1) * 512]
                        nc.tensor.matmul(o, lhs0, mt0[:, j0:j0 + 512],
                                         start=True, stop=False)
                        nc.tensor.matmul(o, lhs1, mt1[:, j0:j0 + 512],
                                         start=False, stop=False)
                        nc.tensor.matmul(o, ones2[:], sq_flat[:, j0:j0 + 512],
                                         start=False, stop=True)
                    if jc == 0:
                        # clamp the i==j block so d2 >= 0 there
                        dg = ps[:, r * 128:(r + 1) * 128]
                        nc.vector.tensor_scalar_min(dg, dg, half_own[:, r:r + 1])
                    out_t = stage_pool.tile([128, JCHUNK], F32, tag="stage")
                    nc.scalar.activation(
                        out_t[:], ps[:],
                        mybir.ActivationFunctionType.Sqrt,
                        bias=sq_tiles[:, r:r + 1], scale=-2.0,
                    )
                    nc.sync.dma_start(
                        out_d[r * 128:(r + 1) * 128,
                              jc * JCHUNK:(jc + 1) * JCHUNK],
                        out_t[:],
                    )


_NC_CACHE = None


def _get_nc():
    global _NC_CACHE
    if _NC_CACHE is None:
        _NC_CACHE = _build_nc()
    return _NC_CACHE


def kernel(mapping: np.ndarray, **_kwargs) -> np.ndarray:
    mapping = np.asarray(mapping, dtype=np.float32)
    assert mapping.shape == (N, D)
    xh = mapping.astype(np.float16)
    eye = np.eye(128, dtype=np.float32)

    in_maps = []
    for c in range(NCORES):
        natc = np.ascontiguousarray(np.roll(xh, -c * RPC, axis=0))
        mtc = np.ascontiguousarray(natc.T)
        in_maps.append({"mt": mtc, "nat": natc, "eye": eye})

    nc = _get_nc()
    res = run_bass_kernel_spmd(nc, in_maps, core_ids=list(range(NCORES)))

    out = np.empty((N, N), dtype=np.float32)
    for c in range(NCORES):
        out[c * RPC:(c + 1) * RPC] = np.roll(res.results[c]["out"], c * RPC, axis=1)
    return out


if __name__ == "__main__":
    rng = np.random.default_rng(0)
    x = rng.standard_normal((N, D)).astype(np.float32)
    o = kernel(mapping=x)
    print("out", o.shape, o.dtype, "sample", o[0, :4], "diag", np.abs(np.diag(o)).max())
